# revision 9
# baseline (speedup 1.0000x reference)
"""Multi-head attention (B=1, H=64, S=362, D=506) with softmax + dropout(p=0.1,
train, jax key 42) on 8 trn2 NeuronCores, head-parallel (8 heads per core).

Transposed-scores design: the device computes scores_T[k, q] = (K Q^T)/sqrt(D)
+ mask_T, exp, dropout — so the attention matrix comes out of softmax already
in the layout the AV matmul needs as its moving operand (no on-chip
transposes). The softmax denominator is a ones-vector matmul on the PE;
the final division by it (and the output [D,S] -> [S,D] transpose) happens
on the host, which also pre-transposes Q/K to [D, S], folds 1/sqrt(D) into
Q and the 1/(1-p) dropout scale into V, pads D->512 / S->384 so every DMA
is one large contiguous transfer, and precomputes the jax threefry dropout
keep-mask as {0,1} uint8.

DMA loads are split across both HWDGE rings (sync + scalar); stores go
through SWDGE (gpsimd).
"""

import math
from contextlib import ExitStack

import numpy as np

import concourse.bass as bass
import concourse.mybir as mybir
import concourse.tile as tile
from concourse import bacc
from concourse.bass_utils import run_bass_kernel_spmd

B, H, S, D = 1, 64, 362, 506
SP = 384  # S padded to 3*128
DP = 512  # D padded to 4*128
N_CORES = 8
HPC = H // N_CORES  # heads per core
DROP_P = 0.1

F32 = mybir.dt.float32
U8 = mybir.dt.uint8

# I/O + matmul dtype mode: "f32" (exact, 4 cyc/row), "f32r" (full rate,
# TF32-ish multiply), "bf16" (full rate + half DMA). Set before first call.
MM_MODE = "f32r"
TRACE = False
LAST_RESULTS = None

_K_CHUNKS = [(0, 128), (128, 128), (256, 106)]  # S = 362 (pad 384)
_D_CHUNKS = [(0, 128), (128, 128), (256, 128), (384, 122)]  # D = 506 (pad 512)

_CACHE = {}


def _mm_dt():
    return {
        "f32": mybir.dt.float32,
        "f32r": mybir.dt.float32r,
        "bf16": mybir.dt.bfloat16,
    }[MM_MODE]


def _np_io_dt():
    import ml_dtypes

    return {"f32": np.float32, "f32r": np.float32, "bf16": ml_dtypes.bfloat16}[MM_MODE]


def _build_nc():
    MM = _mm_dt()
    nc = bacc.Bacc(
        "TRN2",
        target_bir_lowering=False,
        debug=False,
        num_devices=N_CORES,
    )
    qT = nc.dram_tensor("qT", [HPC, DP, S], MM, kind="ExternalInput").ap()
    kT = nc.dram_tensor("kT", [HPC, DP, S], MM, kind="ExternalInput").ap()
    v = nc.dram_tensor("v", [HPC, SP, D], MM, kind="ExternalInput").ap()
    mskT = nc.dram_tensor("mskT", [SP, S], F32, kind="ExternalInput").ap()
    keepT = nc.dram_tensor("keepT", [HPC, SP, S], U8, kind="ExternalInput").ap()
    outT = nc.dram_tensor("outT", [HPC, DP, S], F32, kind="ExternalOutput").ap()
    sums = nc.dram_tensor("sums", [1, HPC, S], F32, kind="ExternalOutput").ap()

    with tile.TileContext(nc) as tc, ExitStack() as ctx:
        const = ctx.enter_context(tc.tile_pool(name="const", bufs=1))
        io = ctx.enter_context(tc.tile_pool(name="io", bufs=2))
        work = ctx.enter_context(tc.tile_pool(name="work", bufs=2))
        ps_s = ctx.enter_context(tc.tile_pool(name="ps_s", bufs=3, space="PSUM"))
        ps_sum = ctx.enter_context(tc.tile_pool(name="ps_sum", bufs=2, space="PSUM"))
        ps_o = ctx.enter_context(tc.tile_pool(name="ps_o", bufs=3, space="PSUM"))

        ones_f32 = const.tile([128, 1], F32, tag="ones_f32")
        nc.gpsimd.memset(ones_f32[:], 1.0)
        if MM == F32:
            ones = ones_f32
        else:
            ones = const.tile([128, 1], MM, tag="ones")
            nc.scalar.copy(ones[:], ones_f32[:])

        # attn mask (transposed), one batched load: [k-part, kc, q]
        msk_sb = const.tile([128, 3, S], F32, tag="msk")
        nc.sync.dma_start(msk_sb[:], mskT.rearrange("(c p) s -> p c s", p=128))

        # all heads' softmax sums accumulate here, one store at the end
        sums_sb = const.tile([1, HPC, S], F32, tag="sums")

        for h in range(HPC):
            # ---- loads (one large DMA per tensor per head) ----
            qt = io.tile([128, 4, S], MM, tag="qt", bufs=3)
            nc.sync.dma_start(qt[:], qT[h].rearrange("(c p) s -> p c s", p=128))
            kt = io.tile([128, 4, S], MM, tag="kt", bufs=3)
            nc.scalar.dma_start(kt[:], kT[h].rearrange("(c p) s -> p c s", p=128))
            vv = io.tile([128, 3, D], MM, tag="v", bufs=3)
            nc.sync.dma_start(vv[:], v[h].rearrange("(c p) d -> p c d", p=128))
            kp = io.tile([128, 3, S], U8, tag="keep", bufs=3)
            nc.scalar.dma_start(kp[:], keepT[h].rearrange("(c p) s -> p c s", p=128))

            # ---- scores_T + softmax-numerator + dropout ----
            sum_ps = ps_sum.tile([1, S], F32, tag="sum")
            att = []
            for kc, (ks, ksz) in enumerate(_K_CHUNKS):
                ps = ps_s.tile([ksz, S], F32, tag="ps")
                for dc in range(4):
                    nc.tensor.matmul(
                        ps[:],
                        kt[:, dc, ks : ks + ksz],
                        qt[:, dc, :],
                        start=(dc == 0),
                        stop=(dc == 3),
                    )
                # scores += mask_T (in place in PSUM)
                nc.vector.tensor_tensor(
                    ps[:], ps[:], msk_sb[:ksz, kc, :], op=mybir.AluOpType.add
                )
                e = work.tile([ksz, S], MM, tag="e", bufs=4)
                nc.scalar.activation(e[:], ps[:], mybir.ActivationFunctionType.Exp)
                # denominator: ones^T @ e accumulated over k chunks
                nc.tensor.matmul(
                    sum_ps[:],
                    ones[:ksz, :],
                    e[:],
                    start=(kc == 0),
                    stop=(kc == len(_K_CHUNKS) - 1),
                )
                a = work.tile([ksz, S], MM, tag="a", bufs=6)
                nc.vector.tensor_tensor(
                    a[:], e[:], kp[:ksz, kc, :], op=mybir.AluOpType.mult
                )
                att.append(a)
            nc.vector.tensor_copy(sums_sb[:, h, :], sum_ps[:])

            # ---- AV (out_T[d, q]) + one batched store per head ----
            o = work.tile([128, 4, S], F32, tag="o", bufs=2)
            for dc, (ds, dsz) in enumerate(_D_CHUNKS):
                po = ps_o.tile([dsz, S], F32, tag="po")
                for kc, (ks, ksz) in enumerate(_K_CHUNKS):
                    nc.tensor.matmul(
                        po[:],
                        vv[:ksz, kc, ds : ds + dsz],
                        att[kc][:],
                        start=(kc == 0),
                        stop=(kc == len(_K_CHUNKS) - 1),
                    )
                nc.scalar.copy(o[:dsz, dc, :], po[:])
            nc.gpsimd.dma_start(outT[h].rearrange("(c p) s -> p c s", p=128), o[:])

        nc.gpsimd.dma_start(sums[:], sums_sb[:])

    nc.finalize()
    return nc


def _get_nc():
    key = MM_MODE
    if key not in _CACHE:
        _CACHE[key] = _build_nc()
    return _CACHE[key]


def _keep_mask():
    """Dropout keep mask, identical bits to the reference (threefry is
    backend-deterministic); computed on the CPU backend."""
    import jax

    cpu = jax.devices("cpu")[0]
    with jax.default_device(cpu):
        k = jax.random.bernoulli(jax.random.key(42), 1.0 - DROP_P, (B, H, S, S))
        return np.asarray(k)


def kernel(query, key, value, attn_mask):
    global LAST_RESULTS
    io_dt = _np_io_dt()
    q = np.asarray(query, dtype=np.float32)[0]  # [H, S, D]
    k = np.asarray(key, dtype=np.float32)[0]
    v = np.asarray(value, dtype=np.float32)[0]
    mskT = np.zeros((SP, S), dtype=np.float32)
    mskT[:S] = np.asarray(attn_mask, dtype=np.float32)[0, 0].T

    scale = 1.0 / math.sqrt(D)
    qT = np.zeros((H, DP, S), dtype=io_dt)
    qT[:, :D] = (np.transpose(q, (0, 2, 1)) * scale).astype(io_dt)
    kT = np.zeros((H, DP, S), dtype=io_dt)
    kT[:, :D] = np.transpose(k, (0, 2, 1)).astype(io_dt)
    vp = np.zeros((H, SP, D), dtype=io_dt)
    vp[:, :S] = (v * np.float32(1.0 / (1.0 - DROP_P))).astype(io_dt)
    keepu = np.zeros((H, SP, S), dtype=np.uint8)
    keepu[:, :S] = np.transpose(_keep_mask()[0], (0, 2, 1))

    nc = _get_nc()
    in_maps = []
    for c in range(N_CORES):
        sl = slice(c * HPC, (c + 1) * HPC)
        in_maps.append(
            {
                "qT": np.ascontiguousarray(qT[sl]),
                "kT": np.ascontiguousarray(kT[sl]),
                "v": np.ascontiguousarray(vp[sl]),
                "mskT": mskT,
                "keepT": np.ascontiguousarray(keepu[sl]),
            }
        )

    res = run_bass_kernel_spmd(nc, in_maps, list(range(N_CORES)), trace=TRACE)
    LAST_RESULTS = res
    outT = np.concatenate(
        [res.results[c]["outT"][:, :D, :] for c in range(N_CORES)], axis=0
    )  # [H, D, S]
    sums = np.concatenate(
        [res.results[c]["sums"][0] for c in range(N_CORES)], axis=0
    )  # [H, S]
    out = np.transpose(outT, (0, 2, 1)) / sums[:, :, None]
    return out.reshape(B, H, S, D).astype(np.float32)


# revision 10
# speedup vs baseline: 1.1949x; 1.1949x over previous
"""Multi-head attention (B=1, H=64, S=362, D=506) with softmax + dropout(p=0.1,
train, jax key 42) on 8 trn2 NeuronCores, head-parallel (8 heads per core).

Transposed-scores design: the device computes scores_T[k, q] = (K Q^T)/sqrt(D)
+ mask_T, exp, dropout — so the attention matrix comes out of softmax already
in the layout the AV matmul needs as its moving operand (no on-chip
transposes). The softmax denominator is a ones-vector matmul on the PE;
the final division by it (and the output [D,S] -> [S,D] transpose) happens
on the host, which also pre-transposes Q/K to [D, S], folds 1/sqrt(D) into
Q and the 1/(1-p) dropout scale into V, pads D->512 / S->384 so every DMA
is one large contiguous transfer, and precomputes the jax threefry dropout
keep-mask as {0,1} uint8.

DMA loads are split across both HWDGE rings (sync + scalar); stores go
through SWDGE (gpsimd).
"""

import math
from contextlib import ExitStack

import numpy as np

import concourse.bass as bass
import concourse.mybir as mybir
import concourse.tile as tile
from concourse import bacc
from concourse.bass_utils import run_bass_kernel_spmd

B, H, S, D = 1, 64, 362, 506
SP = 384  # S padded to 3*128
DP = 512  # D padded to 4*128
N_CORES = 8
HPC = H // N_CORES  # heads per core
DROP_P = 0.1

F32 = mybir.dt.float32
U8 = mybir.dt.uint8

# I/O + matmul dtype mode: "f32" (exact, 4 cyc/row), "f32r" (full rate,
# TF32-ish multiply), "bf16" (full rate + half DMA). Set before first call.
MM_MODE = "f32r"
TRACE = False
LAST_RESULTS = None

_K_CHUNKS = [(0, 128), (128, 128), (256, 106)]  # S = 362 (pad 384)
_D_CHUNKS = [(0, 128), (128, 128), (256, 128), (384, 122)]  # D = 506 (pad 512)

_CACHE = {}


def _mm_dt():
    return {
        "f32": mybir.dt.float32,
        "f32r": mybir.dt.float32r,
        "bf16": mybir.dt.bfloat16,
    }[MM_MODE]


def _np_io_dt():
    import ml_dtypes

    return {"f32": np.float32, "f32r": np.float32, "bf16": ml_dtypes.bfloat16}[MM_MODE]


def _build_nc():
    MM = _mm_dt()
    nc = bacc.Bacc(
        "TRN2",
        target_bir_lowering=False,
        debug=False,
        num_devices=N_CORES,
    )
    qT = nc.dram_tensor("qT", [HPC, DP, S], MM, kind="ExternalInput").ap()
    kT = nc.dram_tensor("kT", [HPC, DP, S], MM, kind="ExternalInput").ap()
    v = nc.dram_tensor("v", [HPC, SP, D], MM, kind="ExternalInput").ap()
    mskT = nc.dram_tensor("mskT", [SP, S], F32, kind="ExternalInput").ap()
    keepT = nc.dram_tensor("keepT", [HPC, SP, S], U8, kind="ExternalInput").ap()
    outT = nc.dram_tensor("outT", [HPC, DP, S], F32, kind="ExternalOutput").ap()
    sums = nc.dram_tensor("sums", [1, HPC, S], F32, kind="ExternalOutput").ap()

    with tile.TileContext(nc) as tc, ExitStack() as ctx:
        const = ctx.enter_context(tc.tile_pool(name="const", bufs=1))
        io = ctx.enter_context(tc.tile_pool(name="io", bufs=2))
        work = ctx.enter_context(tc.tile_pool(name="work", bufs=2))
        ps_s = ctx.enter_context(tc.tile_pool(name="ps_s", bufs=3, space="PSUM"))
        ps_sum = ctx.enter_context(tc.tile_pool(name="ps_sum", bufs=2, space="PSUM"))
        ps_o = ctx.enter_context(tc.tile_pool(name="ps_o", bufs=3, space="PSUM"))

        ones_f32 = const.tile([128, 1], F32, tag="ones_f32")
        nc.gpsimd.memset(ones_f32[:], 1.0)
        if MM == F32:
            ones = ones_f32
        else:
            ones = const.tile([128, 1], MM, tag="ones")
            nc.scalar.copy(ones[:], ones_f32[:])

        # attn mask (transposed), one batched load: [k-part, kc, q]
        msk_sb = const.tile([128, 3, S], F32, tag="msk")
        nc.sync.dma_start(msk_sb[:], mskT.rearrange("(c p) s -> p c s", p=128))

        # all heads' softmax sums accumulate here, one store at the end
        sums_sb = const.tile([1, HPC, S], F32, tag="sums")

        for h in range(HPC):
            # ---- loads (one large DMA per tensor per head) ----
            qt = io.tile([128, 4, S], MM, tag="qt", bufs=3)
            nc.sync.dma_start(qt[:], qT[h].rearrange("(c p) s -> p c s", p=128))
            kt = io.tile([128, 4, S], MM, tag="kt", bufs=3)
            nc.sync.dma_start(kt[:], kT[h].rearrange("(c p) s -> p c s", p=128))
            vv = io.tile([128, 3, D], MM, tag="v", bufs=3)
            nc.gpsimd.dma_start(vv[:], v[h].rearrange("(c p) d -> p c d", p=128))
            kp = io.tile([128, 3, S], U8, tag="keep", bufs=3)
            nc.scalar.dma_start(kp[:], keepT[h].rearrange("(c p) s -> p c s", p=128))

            # ---- scores_T + exp (QK matmuls stay back-to-back on PE) ----
            es = []
            for kc, (ks, ksz) in enumerate(_K_CHUNKS):
                ps = ps_s.tile([ksz, S], F32, tag="ps")
                for dc in range(4):
                    nc.tensor.matmul(
                        ps[:],
                        kt[:, dc, ks : ks + ksz],
                        qt[:, dc, :],
                        start=(dc == 0),
                        stop=(dc == 3),
                    )
                # scores += mask_T (in place in PSUM)
                nc.vector.tensor_tensor(
                    ps[:], ps[:], msk_sb[:ksz, kc, :], op=mybir.AluOpType.add
                )
                e = work.tile([ksz, S], MM, tag="e", bufs=4)
                nc.scalar.activation(e[:], ps[:], mybir.ActivationFunctionType.Exp)
                es.append(e)

            # denominator: ones^T @ e, emitted after all QK so the PE
            # doesn't stall on the DVE/ACT chain of each chunk
            sum_ps = ps_sum.tile([1, S], F32, tag="sum")
            for kc, (ks, ksz) in enumerate(_K_CHUNKS):
                nc.tensor.matmul(
                    sum_ps[:],
                    ones[:ksz, :],
                    es[kc][:],
                    start=(kc == 0),
                    stop=(kc == len(_K_CHUNKS) - 1),
                )

            # dropout
            att = []
            for kc, (ks, ksz) in enumerate(_K_CHUNKS):
                a = work.tile([ksz, S], MM, tag="a", bufs=6)
                nc.vector.tensor_tensor(
                    a[:], es[kc][:], kp[:ksz, kc, :], op=mybir.AluOpType.mult
                )
                att.append(a)
            nc.vector.tensor_copy(sums_sb[:, h, :], sum_ps[:])

            # ---- AV (out_T[d, q]) + one batched store per head ----
            o = work.tile([128, 4, S], F32, tag="o", bufs=2)
            for dc, (ds, dsz) in enumerate(_D_CHUNKS):
                po = ps_o.tile([dsz, S], F32, tag="po")
                for kc, (ks, ksz) in enumerate(_K_CHUNKS):
                    nc.tensor.matmul(
                        po[:],
                        vv[:ksz, kc, ds : ds + dsz],
                        att[kc][:],
                        start=(kc == 0),
                        stop=(kc == len(_K_CHUNKS) - 1),
                    )
                nc.scalar.copy(o[:dsz, dc, :], po[:])
            nc.gpsimd.dma_start(outT[h].rearrange("(c p) s -> p c s", p=128), o[:])

        nc.gpsimd.dma_start(sums[:], sums_sb[:])

    nc.finalize()
    return nc


def _get_nc():
    key = MM_MODE
    if key not in _CACHE:
        _CACHE[key] = _build_nc()
    return _CACHE[key]


def _keep_mask():
    """Dropout keep mask, identical bits to the reference (threefry is
    backend-deterministic); computed on the CPU backend."""
    import jax

    cpu = jax.devices("cpu")[0]
    with jax.default_device(cpu):
        k = jax.random.bernoulli(jax.random.key(42), 1.0 - DROP_P, (B, H, S, S))
        return np.asarray(k)


def kernel(query, key, value, attn_mask):
    global LAST_RESULTS
    io_dt = _np_io_dt()
    q = np.asarray(query, dtype=np.float32)[0]  # [H, S, D]
    k = np.asarray(key, dtype=np.float32)[0]
    v = np.asarray(value, dtype=np.float32)[0]
    mskT = np.zeros((SP, S), dtype=np.float32)
    mskT[:S] = np.asarray(attn_mask, dtype=np.float32)[0, 0].T

    scale = 1.0 / math.sqrt(D)
    qT = np.zeros((H, DP, S), dtype=io_dt)
    qT[:, :D] = (np.transpose(q, (0, 2, 1)) * scale).astype(io_dt)
    kT = np.zeros((H, DP, S), dtype=io_dt)
    kT[:, :D] = np.transpose(k, (0, 2, 1)).astype(io_dt)
    vp = np.zeros((H, SP, D), dtype=io_dt)
    vp[:, :S] = (v * np.float32(1.0 / (1.0 - DROP_P))).astype(io_dt)
    keepu = np.zeros((H, SP, S), dtype=np.uint8)
    keepu[:, :S] = np.transpose(_keep_mask()[0], (0, 2, 1))

    nc = _get_nc()
    in_maps = []
    for c in range(N_CORES):
        sl = slice(c * HPC, (c + 1) * HPC)
        in_maps.append(
            {
                "qT": np.ascontiguousarray(qT[sl]),
                "kT": np.ascontiguousarray(kT[sl]),
                "v": np.ascontiguousarray(vp[sl]),
                "mskT": mskT,
                "keepT": np.ascontiguousarray(keepu[sl]),
            }
        )

    res = run_bass_kernel_spmd(nc, in_maps, list(range(N_CORES)), trace=TRACE)
    LAST_RESULTS = res
    outT = np.concatenate(
        [res.results[c]["outT"][:, :D, :] for c in range(N_CORES)], axis=0
    )  # [H, D, S]
    sums = np.concatenate(
        [res.results[c]["sums"][0] for c in range(N_CORES)], axis=0
    )  # [H, S]
    out = np.transpose(outT, (0, 2, 1)) / sums[:, :, None]
    return out.reshape(B, H, S, D).astype(np.float32)
